# revision 1
# baseline (speedup 1.0000x reference)
"""APPNP GNN (nn_APPNPNet) distributed Trainium2 kernel - 8 NeuronCores.

Math (PyG APPNP, eval mode):
    deg[d]  = in-degree(d) + 1 (self loop);  dinv = deg^-1/2
    h       = relu(x @ W1 + b1) @ W2 + b2;  z_0 = h
    z_{k+1} = (1-a) * dinv (.) ( agg + y ) + a*h,   y = dinv (.) z,
              agg[d] = sum_{e->d} y[src[e]]
    out     = z_K

Device state: w = dinv (.) z, so the shared/gathered quantity IS w:
    w_{k+1} = (1-a)/deg (.) ( agg + w ) + a*dinv (.) h,  agg[d] = sum w[src]
    z_K     = sqrt(deg) (.) w_K               (host-side rescale)

Sharding: dst-node rows, 12544/core (12500 real + 44 pad), nodes permuted
by descending in-degree so each 128-node tile has near-uniform in-degree.
Incoming edges live in fixed-width slot grids, split by source-gid block
(dma_gather has int16 indices -> 4 blocks of 32768 table rows).  Layout is
slab-major: for each source block b, tiles are chunked with uniform width
W_b; the segment-sum is a dense in-place tree reduction over each slab
chunk (vector engine), accumulated into agg; no scatter anywhere.
"""

import numpy as np

import concourse.bass as bass
import concourse.bacc as bacc
import concourse.tile as tile
from concourse import mybir
from concourse.bass_utils import run_bass_kernel_spmd
from concourse.masks import make_identity

# ---- problem constants (hardcoded; kernel.py must be self-contained) ----
N = 100000
E = 3200000
IN_C, HID_C, OUT_C = 512, 256, 64
K = 10
ALPHA = 0.1

NCORES = 8
SH_REAL = N // NCORES          # 12500 real nodes per core
P = 128
NTILES = 98
SH = NTILES * P                # 12544 rows per core
F = OUT_C                      # 64
GV = SH * NCORES               # 100352 rows in the gathered table
BLK = 32768                    # dma_gather int16 index range per block
NBLK = 4
# an always-zero row inside each block (core pad regions):
PAD_GID = [12500, 2 * SH + 12500, 5 * SH + 12500, 7 * SH + 12500]

# tuning knobs
COLS_MAX = 96                  # max slot-columns per chunk buffer
CALL_MAX = 4096                # max indices per dma_gather call
NQUEUES = 4
F32 = mybir.dt.float32
BF = mybir.dt.bfloat16


def _to_bf16(a: np.ndarray) -> np.ndarray:
    import ml_dtypes
    return a.astype(ml_dtypes.bfloat16)


def install_dge_levels_patch():
    """walrus defaults exclude the dynamic-DMA dge levels and use a small
    SWDGE descriptor carveout; patch both into every codegen call."""
    import concourse.bass_utils as bu
    if getattr(bu, "_dge_patched", False):
        return
    _orig = bu.run_command

    def run_patched(cmd, **kw):
        if cmd and "walrus_driver" in str(cmd[0]) and "--pass" in cmd:
            cmd = list(cmd) + [
                "--dge-levels=io,spill_reload,scalar_dynamic_offset,"
                "vector_dynamic_offsets,dynamic_size,dst_reduce",
                "--dynamic-dma-scratch-size-per-partition=8192",
            ]
        return _orig(cmd, **kw)

    bu.run_command = run_patched
    bu._dge_patched = True


# ---------------------------------------------------------------------------
# host-side graph preprocessing
# ---------------------------------------------------------------------------

def preprocess(x, edge_index, W1, b1, W2, b2):
    src = np.asarray(edge_index[0], dtype=np.int64)
    dst = np.asarray(edge_index[1], dtype=np.int64)

    deg = np.bincount(dst, minlength=N).astype(np.float64) + 1.0
    dinv = (1.0 / np.sqrt(deg)).astype(np.float32)
    indeg = np.bincount(dst, minlength=N)

    perm = np.empty((NCORES, SH_REAL), dtype=np.int64)    # pos -> orig local
    gid_of_node = np.empty(N, dtype=np.int64)
    for c in range(NCORES):
        lo = c * SH_REAL
        order = np.argsort(-indeg[lo:lo + SH_REAL], kind="stable")
        perm[c] = order
        inv = np.empty(SH_REAL, dtype=np.int64)
        inv[order] = np.arange(SH_REAL)
        gid_of_node[lo:lo + SH_REAL] = c * SH + inv

    dst_core = dst // SH_REAL
    dst_pos = gid_of_node[dst] - dst_core * SH            # permuted local pos
    src_gid = gid_of_node[src]
    src_blk = src_gid // BLK

    W = np.zeros((NTILES, NBLK), dtype=np.int64)
    core_tables = []
    for c in range(NCORES):
        m = dst_core == c
        e_pos = dst_pos[m]
        e_src = src_gid[m]
        e_blk = src_blk[m]
        o = np.lexsort((e_src, e_blk, e_pos))
        e_pos, e_src, e_blk = e_pos[o], e_src[o], e_blk[o]
        cnt = np.zeros((SH, NBLK), dtype=np.int64)
        np.add.at(cnt, (e_pos, e_blk), 1)
        starts = np.zeros((SH, NBLK), dtype=np.int64)
        starts.reshape(-1)[1:] = np.cumsum(cnt.reshape(-1))[:-1]
        W = np.maximum(W, cnt.reshape(NTILES, P, NBLK).max(axis=1))
        core_tables.append((cnt, starts, e_src))

    # slab chunk plans: per block, group consecutive tiles, uniform width
    plans = []
    for b in range(NBLK):
        plan = []
        t = 0
        while t < NTILES:
            wd = int(W[t, b])
            if wd == 0:
                nt = 1
                while t + nt < NTILES and W[t + nt, b] == 0:
                    nt += 1
                plan.append((t, nt, 0))
                t += nt
                continue
            nt = 1
            while (
                t + nt < NTILES
                and (nt + 1) * max(wd, int(W[t + nt, b])) <= COLS_MAX
                and W[t + nt, b] > 0
            ):
                wd = max(wd, int(W[t + nt, b]))
                nt += 1
            plan.append((t, nt, wd))
            t += nt
        plans.append(plan)

    # call list: (block, t0, nt, wd, idx16_off, n_idx), nt*wd*P <= CALL_MAX
    calls = []
    off16 = 0
    for b in range(NBLK):
        for (t0, nt, wd) in plans[b]:
            if wd == 0:
                continue
            j = 0
            while j < nt:
                step_nt = max(1, min(nt - j, CALL_MAX // (wd * P)))
                nidx = step_nt * wd * P
                calls.append((b, t0 + j, step_nt, wd, off16, nidx))
                off16 += nidx // 16
                j += step_nt
    totc16 = off16

    in_maps = []
    for c in range(NCORES):
        cnt, starts, e_src = core_tables[c]
        gidx = np.empty((P, totc16), dtype=np.int16)
        for (b, t0, nt, wd, o16, nidx) in calls:
            arr = np.full(nidx, PAD_GID[b] - b * BLK, dtype=np.int16)
            for tl in range(nt):
                tile_i = t0 + tl
                base = tl * wd * P
                for p in range(P):
                    pos = tile_i * P + p
                    k = cnt[pos, b]
                    if k:
                        s = starts[pos, b]
                        arr[base + p: base + k * P + p: P] = \
                            e_src[s:s + k] - b * BLK
            w16 = arr.reshape(-1, 16).T
            gidx[:, o16:o16 + nidx // 16] = np.tile(w16, (8, 1))

        lo = c * SH_REAL
        pm = perm[c]
        xt = np.zeros((IN_C, SH), dtype=np.float32)
        xt[:, :SH_REAL] = np.asarray(x[lo:lo + SH_REAL], np.float32).T[:, pm]
        dloc = np.zeros(SH, dtype=np.float32)
        dloc[:SH_REAL] = dinv[lo:lo + SH_REAL][pm]
        dl = dloc.reshape(NTILES, P).T.copy()
        degloc = np.ones(SH, dtype=np.float64)
        degloc[:SH_REAL] = deg[lo:lo + SH_REAL][pm]
        s19 = ((1.0 - ALPHA) / degloc).astype(np.float32)
        s19[SH_REAL:] = 0.0
        s19x = np.repeat(s19.reshape(NTILES, P).T[:, :, None], F, axis=2)
        in_maps.append({
            "xt": _to_bf16(xt),
            "w1": _to_bf16(np.asarray(W1, np.float32)),
            "w2": _to_bf16(np.asarray(W2, np.float32)),
            "b1": np.asarray(b1, np.float32).reshape(HID_C, 1),
            "b2": np.asarray(b2, np.float32).reshape(OUT_C, 1),
            "dnm": dl,
            "adnm": (ALPHA * dl).astype(np.float32),
            "s19x": np.ascontiguousarray(s19x.reshape(P, NTILES * F)),
            "gidx": gidx,
        })

    sqrt_deg = np.sqrt(deg).astype(np.float32)
    return in_maps, calls, totc16, perm, sqrt_deg


# ---------------------------------------------------------------------------
# device program
# ---------------------------------------------------------------------------

def build_program(calls, totc16):
    nc = bacc.Bacc(None, target_bir_lowering=False, debug=False,
                   num_devices=NCORES, num_swdge_queues=NQUEUES)

    xt = nc.declare_dram_parameter("xt", [IN_C, SH], BF, isOutput=False)
    w1 = nc.declare_dram_parameter("w1", [IN_C, HID_C], BF, isOutput=False)
    w2 = nc.declare_dram_parameter("w2", [HID_C, OUT_C], BF, isOutput=False)
    b1 = nc.declare_dram_parameter("b1", [HID_C, 1], F32, isOutput=False)
    b2 = nc.declare_dram_parameter("b2", [OUT_C, 1], F32, isOutput=False)
    dnm = nc.declare_dram_parameter("dnm", [P, NTILES], F32, isOutput=False)
    adnm = nc.declare_dram_parameter("adnm", [P, NTILES], F32, isOutput=False)
    s19x = nc.declare_dram_parameter("s19x", [P, NTILES * F], F32, isOutput=False)
    gidx_d = nc.declare_dram_parameter("gidx", [P, totc16], mybir.dt.int16,
                                       isOutput=False)
    out = nc.declare_dram_parameter("out", [SH, F], F32, isOutput=True)

    ybounce = nc.dram_tensor("ybounce", [SH, F], F32)
    gfull = nc.dram_tensor("gfull", [GV, F], F32)

    FREE = NTILES * F

    with tile.TileContext(nc) as tc:
        with (
            tc.tile_pool(name="persist", bufs=1) as pp,
            tc.tile_pool(name="mlpx", bufs=2) as xp,
            tc.tile_pool(name="mlphh", bufs=2) as hp,
            tc.tile_pool(name="psum", bufs=2, space="PSUM") as psp,
            tc.tile_pool(name="psumT", bufs=2, space="PSUM") as pst,
            tc.tile_pool(name="gather", bufs=3) as gp,
            tc.tile_pool(name="gidxp", bufs=3) as ixp,
        ):
            wsb = pp.tile([P, FREE], F32, tag="w")
            agg = pp.tile([P, FREE], F32, tag="agg")
            ahd = pp.tile([P, FREE], F32, tag="ahd")
            s19 = pp.tile([P, FREE], F32, tag="s19")
            dnt = pp.tile([P, NTILES], F32, tag="dnt")
            adt = pp.tile([P, NTILES], F32, tag="adt")
            ident = pp.tile([P, P], F32, tag="ident")
            w1s = pp.tile([P, 4 * HID_C], BF, tag="w1s")
            w2s = pp.tile([P, 2 * OUT_C], BF, tag="w2s")
            b1s = pp.tile([P, 2], F32, tag="b1s")
            b2s = pp.tile([P, 1], F32, tag="b2s")

            make_identity(nc, ident[:])
            nc.sync.dma_start(out=s19[:], in_=s19x[:])
            nc.sync.dma_start(out=dnt[:], in_=dnm[:])
            nc.sync.dma_start(out=adt[:], in_=adnm[:])
            nc.sync.dma_start(
                out=w1s[:].rearrange("p (k h) -> p k h", k=4),
                in_=w1.ap().rearrange("(k p) h -> p k h", p=P))
            nc.sync.dma_start(
                out=w2s[:].rearrange("p (k o) -> p k o", k=2),
                in_=w2.ap().rearrange("(k p) o -> p k o", p=P))
            nc.sync.dma_start(
                out=b1s[:].rearrange("p (k o) -> p k o", k=2),
                in_=b1.ap().rearrange("(k p) o -> p k o", p=P))
            nc.sync.dma_start(out=b2s[:OUT_C, :], in_=b2.ap())

            # ---- MLP (feature-major) + transpose into node-major w0/ahd ----
            with nc.named_scope("mlp"):
                nchunks = [(i * 512, min(512, SH - i * 512))
                           for i in range((SH + 511) // 512)]
                for (n0, ncols) in nchunks:
                    xts = [xp.tile([P, 512], BF, tag=f"xt{k}", name=f"xt{k}")
                           for k in range(4)]
                    for kc in range(4):
                        nc.sync.dma_start(
                            out=xts[kc][:, :ncols],
                            in_=xt.ap()[kc * P:(kc + 1) * P, n0:n0 + ncols])
                    hhs = []
                    for half in range(2):
                        ps1 = psp.tile([P, 512], F32, tag="ps1", name="ps1")
                        for kc in range(4):
                            nc.tensor.matmul(
                                ps1[:, :ncols],
                                lhsT=w1s[:, kc * HID_C + half * P:
                                         kc * HID_C + half * P + P],
                                rhs=xts[kc][:, :ncols],
                                start=(kc == 0), stop=(kc == 3))
                        hh = hp.tile([P, 512], BF, tag=f"hh{half}", name="hh")
                        nc.scalar.activation(
                            hh[:, :ncols], ps1[:, :ncols],
                            mybir.ActivationFunctionType.Relu,
                            bias=b1s[:, half:half + 1])
                        hhs.append(hh)
                    ps2 = psp.tile([P, 512], F32, tag="ps2", name="ps2")
                    for half in range(2):
                        nc.tensor.matmul(
                            ps2[:OUT_C, :ncols],
                            lhsT=w2s[:, half * OUT_C:(half + 1) * OUT_C],
                            rhs=hhs[half][:, :ncols],
                            start=(half == 0), stop=(half == 1))
                    hts = hp.tile([P, 512], F32, tag="ht", name="hts")
                    nc.scalar.activation(
                        hts[:OUT_C, :ncols], ps2[:OUT_C, :ncols],
                        mybir.ActivationFunctionType.Identity,
                        bias=b2s[:OUT_C, :])
                    for j in range(ncols // P):
                        t_i = (n0 + j * P) // P
                        pt = pst.tile([P, OUT_C], F32, tag="pt", name="pt")
                        nc.tensor.transpose(
                            out=pt[:],
                            in_=hts[:OUT_C, j * P:(j + 1) * P],
                            identity=ident[:OUT_C, :OUT_C])
                        nc.scalar.activation(
                            wsb[:, t_i * F:(t_i + 1) * F], pt[:],
                            mybir.ActivationFunctionType.Copy,
                            scale=dnt[:, t_i:t_i + 1])
                        nc.scalar.activation(
                            ahd[:, t_i * F:(t_i + 1) * F], pt[:],
                            mybir.ActivationFunctionType.Copy,
                            scale=adt[:, t_i:t_i + 1])

            # ---- K propagation steps ----
            ybv = ybounce.ap().rearrange("(t p) f -> p t f", p=P)
            qn = 0
            for step in range(K):
                with nc.named_scope(f"s{step}_share"):
                    nc.sync.dma_start(
                        out=ybv,
                        in_=wsb[:].rearrange("p (t f) -> p t f", t=NTILES))
                    nc.gpsimd.collective_compute(
                        "AllGather", mybir.AluOpType.bypass,
                        replica_groups=[list(range(NCORES))],
                        ins=[ybounce.ap().opt()],
                        outs=[gfull.ap().opt()],
                    )
                with nc.named_scope(f"s{step}_agg"):
                    # agg starts at w (the self-loop term)
                    nc.vector.tensor_copy(out=agg[:], in_=wsb[:])
                    for (b, t0, nt, wd, o16, nidx) in calls:
                        ixt = ixp.tile([P, nidx // 16], mybir.dt.int16,
                                       tag="ix", name="ixt",
                                       padded_shape=[P, CALL_MAX // 16])
                        nc.sync.dma_start(
                            out=ixt[:],
                            in_=gidx_d.ap()[:, o16:o16 + nidx // 16])
                        g = gp.tile([P, nt * wd * F], F32, tag="g", name="g",
                                    padded_shape=[P, COLS_MAX * F])
                        span = min(BLK, GV - b * BLK)
                        nc.gpsimd.dma_gather(
                            out_ap=g[:].rearrange("p (c f) -> p c f", f=F),
                            in_ap=gfull.ap()[b * BLK:b * BLK + span, :],
                            idxs_ap=ixt[:],
                            num_idxs=nidx, num_idxs_reg=nidx, elem_size=F,
                            queue_num=qn % NQUEUES, single_packet=False)
                        qn += 1
                        gv = g[:].rearrange("p (t d f) -> p t (d f)",
                                            t=nt, d=wd)
                        wdt = wd
                        while wdt > 1:
                            if wdt % 2 == 1:
                                nc.vector.tensor_tensor(
                                    out=gv[:, :, :F], in0=gv[:, :, :F],
                                    in1=gv[:, :, (wdt - 1) * F:wdt * F],
                                    op=mybir.AluOpType.add)
                                wdt -= 1
                            h2 = wdt // 2
                            nc.vector.tensor_tensor(
                                out=gv[:, :, :h2 * F], in0=gv[:, :, :h2 * F],
                                in1=gv[:, :, h2 * F:wdt * F],
                                op=mybir.AluOpType.add)
                            wdt = h2
                        av = agg[:, t0 * F:(t0 + nt) * F].rearrange(
                            "p (t f) -> p t f", t=nt)
                        nc.vector.tensor_tensor(
                            out=av, in0=av, in1=gv[:, :, :F],
                            op=mybir.AluOpType.add)
                    # update: w = s19 . agg + ahd
                    nc.vector.tensor_tensor(
                        out=wsb[:], in0=agg[:], in1=s19[:],
                        op=mybir.AluOpType.mult)
                    nc.vector.tensor_tensor(
                        out=wsb[:], in0=wsb[:], in1=ahd[:],
                        op=mybir.AluOpType.add)

            nc.sync.dma_start(
                out=out.ap().rearrange("(t p) f -> p t f", p=P),
                in_=wsb[:].rearrange("p (t f) -> p t f", t=NTILES))

    nc.compile()
    return nc


# ---------------------------------------------------------------------------
# entry point
# ---------------------------------------------------------------------------

_CACHE = {}


def _build_and_run(inputs, trace=False):
    install_dge_levels_patch()
    in_maps, calls, totc16, perm, sqrt_deg = preprocess(**inputs)
    key = (tuple(calls), totc16)
    if key not in _CACHE:
        _CACHE[key] = build_program(calls, totc16)
    nc = _CACHE[key]
    res = run_bass_kernel_spmd(
        nc, in_maps, core_ids=list(range(NCORES)), trace=trace)

    z = np.empty((N, F), dtype=np.float32)
    for c in range(NCORES):
        wc = res.results[c]["out"]
        lo = c * SH_REAL
        z[lo + perm[c]] = wc[:SH_REAL]
    z *= sqrt_deg[:, None]
    return z, res


def kernel(**inputs) -> np.ndarray:
    z, _ = _build_and_run(inputs, trace=False)
    return z

